# revision 19
# baseline (speedup 1.0000x reference)
"""Trainium2 Bass kernel for nn_Net_72447508349482 (dense_cnn).

Pipeline per core (batch sharded 256 -> 8 x 32):
  conv1 (PE matmul over im2col, rhs laid out [J,b,i,j]) -> maxpool2 via one
  DVE XY-reduce (or ACT-assisted copy+max for some row-chunks, for engine
  balance) -> adder2d: |patch - w| via ACT activation(Abs, bias) for ~1/4 of
  the channels and min(patch, w) on DVE tensor_scalar for the rest;
  partition-reduce over k via PE one-hot matmuls, 4-way COLUMN-TILED
  (tile_position=(0,32j)) so four channels' reduces run concurrently in the
  128x128 array. Channels live at permuted PSUM partitions
  p(c) = 32*(c%4) + c//4 (undone for free on the host via permuted FC1/BN
  parameters). -> minpool (= -maxpool(-x), sign folded into BN affine)
  -> BN batch stats with cross-core AllReduce of [128,2] sums
  (rstd via exp(-0.5*ln(var+eps)) keeps ACT on one table set)
  -> FC1 + ReLU -> FC2 -> log_softmax.

conv1 bias is folded into the adder weights on the host:
  maxpool(conv+b) = maxpool(conv)+b and |p + b - w| = |p - (w - b)|.

Self-contained: includes the BIR multi-wait splitting fix required by this
container's walrus build (rejects >1 sync wait per instruction).
"""

import json

import numpy as np

import concourse.bass as bass
import concourse.mybir as mybir
import concourse.tile as tile
from concourse.bass_utils import run_bass_kernel_spmd

N_CORES = 8
B_TOTAL = 256
BC = B_TOTAL // N_CORES          # 32 per core
BN_EPS = 1e-5
F32 = mybir.dt.float32
BF16 = mybir.dt.bfloat16
ALU = mybir.AluOpType
ACTF = mybir.ActivationFunctionType

# channel -> psum partition: four col-tile groups j = c % 4, row r = c // 4
PERM = [32 * (c % 4) + c // 4 for c in range(50)]
# channels whose |p-w| runs on ACT (others: min(p,w) on DVE)
ACT_CH = {c for c in range(50) if c % 4 == 3}
LAST_ROUND = [12, 12, 11, 11]    # last round with a channel, per group


# --------------------------------------------------------------------------
# BIR post-processing: split multi-wait instructions (walrus limit = 1).
# --------------------------------------------------------------------------
def _fix_bir_bytes(raw: bytes) -> bytes:
    d = json.loads(raw)
    for fn in d.get("functions", []):
        for b in fn.get("blocks", []):
            insts = b.get("instructions", [])
            i = 0
            while i < len(insts):
                ins = insts[i]
                si = ins.get("sync_info") or {}
                w = si.get("on_wait") or []
                if len(w) > 1:
                    for j, extra in enumerate(w[:-1]):
                        insts.insert(i, {
                            "name": f"{ins['name']}_wsplit{j}",
                            "opcode": "EventSemaphore",
                            "engine": ins["engine"],
                            "ins": [], "outs": [],
                            "debug": ins.get("debug", 0),
                            "sync_info": {"on_update": [], "on_wait": [extra]},
                        })
                        i += 1
                    si["on_wait"] = w[-1:]
                i += 1
    return json.dumps(d).encode()


def _patch_nc(nc):
    cls_fn = type(nc).to_json_bytes
    nc.to_json_bytes = lambda: _fix_bir_bytes(cls_fn(nc))
    return nc


# --------------------------------------------------------------------------
# Host-side input prep (pure rearranges/folds of the given parameters).
# --------------------------------------------------------------------------
def host_prep(inputs):
    import ml_dtypes
    bf = ml_dtypes.bfloat16
    f = lambda k: np.ascontiguousarray(np.asarray(inputs[k], np.float32))
    x = f("x")                                       # [256,1,28,28]
    conv1_w, conv1_b, adder_w = f("conv1_w"), f("conv1_b"), f("adder_w")
    p = {}
    p["x_im"] = x.reshape(B_TOTAL, 784)
    wfold = adder_w - conv1_b[None, :, None, None]
    A = (-wfold.reshape(50, 20, 25))                 # [o, c, m]
    # chunk q: channels 5q..5q+4; within-chunk partition p = m*5 + c'
    negwT = np.stack(
        [A[:, 5 * q:5 * q + 5, :].transpose(2, 1, 0).reshape(125, 50)
         for q in range(4)], axis=1)                 # [125, 4, 50]
    p["negwT"] = np.ascontiguousarray(negwT)
    poswT = np.stack(
        [(-A)[:, 5 * q:5 * q + 5, :].transpose(2, 1, 0).reshape(125, 50)
         for q in range(4)], axis=1)                 # [125, 4, 50] = +w'
    p["poswT"] = np.ascontiguousarray(poswT)
    p["w1conv"] = np.ascontiguousarray(conv1_w.reshape(20, 25).T).astype(bf)
    oh = np.zeros((125, 64), bf)
    oh[:, 32] = bf(1.0)
    p["onehot32"] = oh
    z2 = np.zeros((125, 64), bf)
    z2[:, 32] = bf(-2.0)
    p["z2_32"] = z2
    ones128 = np.zeros((125, 128), bf)
    for c in range(50):
        if c not in ACT_CH:
            ones128[:, PERM[c]] = bf(1.0)
    p["ones128"] = ones128
    w1 = f("fc1_w").reshape(500, 50, 16)             # [u, c, ij]
    w1p = np.zeros((128, 500, 16), np.float32)
    for c in range(50):
        w1p[PERM[c]] = w1[:, c, :]
    p["w1t"] = np.ascontiguousarray(w1p).astype(bf)  # [128,500,16]
    p["b1t"] = np.ascontiguousarray(f("fc1_b").reshape(4, 125).T)  # [125,4]
    p["w2t"] = np.ascontiguousarray(
        f("fc2_w").T.reshape(4, 125, 10).transpose(1, 0, 2)).astype(bf)
    p["b2"] = f("fc2_b").reshape(10, 1)
    ng = np.zeros((128, 1), np.float32)
    bt = np.zeros((128, 1), np.float32)
    g, b_ = f("bn_gamma"), f("bn_beta")
    for c in range(50):
        ng[PERM[c], 0] = -g[c]
        bt[PERM[c], 0] = b_[c]
    p["neg_gamma"] = ng
    p["beta"] = bt
    p["ident10"] = np.eye(10, dtype=np.float32)
    return p


# --------------------------------------------------------------------------
# Device program.
# --------------------------------------------------------------------------
def build_nc(reps: int = 1, for_sim: bool = False, sim_coltile: bool = False):
    nc = bass.Bass("TRN2", target_bir_lowering=False, debug=False,
                   num_devices=1 if for_sim else N_CORES)
    d_x = nc.dram_tensor("x_imT", [784, BC], BF16, kind="ExternalInput")
    d_w1conv = nc.dram_tensor("w1conv", [25, 20], BF16, kind="ExternalInput")
    d_negwT = nc.dram_tensor("negwT", [125, 4, 50], F32, kind="ExternalInput")
    d_onehot = nc.dram_tensor("onehot32", [125, 64], BF16, kind="ExternalInput")
    d_poswT = nc.dram_tensor("poswT", [125, 4, 50], F32, kind="ExternalInput")
    d_z2 = nc.dram_tensor("z2_32", [125, 64], BF16, kind="ExternalInput")
    d_ones = nc.dram_tensor("ones128", [125, 128], BF16, kind="ExternalInput")
    d_w1t = nc.dram_tensor("w1t", [128, 500, 16], BF16, kind="ExternalInput")
    d_b1t = nc.dram_tensor("b1t", [125, 4], F32, kind="ExternalInput")
    d_w2t = nc.dram_tensor("w2t", [125, 4, 10], BF16, kind="ExternalInput")
    d_b2 = nc.dram_tensor("b2", [10, 1], F32, kind="ExternalInput")
    d_ng = nc.dram_tensor("neg_gamma", [128, 1], F32, kind="ExternalInput")
    d_beta = nc.dram_tensor("beta", [128, 1], F32, kind="ExternalInput")
    d_id10 = nc.dram_tensor("ident10", [10, 10], F32, kind="ExternalInput")
    d_y = nc.dram_tensor("y", [BC, 10], F32, kind="ExternalOutput")
    cc_ins = [nc.dram_tensor(f"cc_in{i}", [128, 2], F32) for i in range(2)]
    cc_outs = [nc.dram_tensor(f"cc_out{i}", [128, 2], F32,
                              addr_space="Shared") for i in range(2)]

    with tile.TileContext(nc) as tc:
        with (
            tc.tile_pool(name="consts", bufs=1) as consts,
            tc.tile_pool(name="big", bufs=1) as big,
            tc.tile_pool(name="work", bufs=3) as work,
            tc.tile_pool(name="absp", bufs=16) as absp,
            tc.tile_pool(name="small", bufs=1) as small,
            tc.tile_pool(name="xrep", bufs=2) as xrep,
        ):
            # ---- load constants (gpsimd SWDGE ring: keeps the sync/scalar
            # HWDGE rings free for the conv feed) ----
            s_w1conv = consts.tile([25, 20], BF16)
            nc.sync.dma_start(out=s_w1conv, in_=d_w1conv[:, :])
            s_negwT = consts.tile([125, 4, 50], F32)
            nc.gpsimd.dma_start(out=s_negwT, in_=d_negwT[:, :, :])
            s_onehot = consts.tile([125, 64], BF16)
            nc.gpsimd.dma_start(out=s_onehot, in_=d_onehot[:, :])
            s_poswT = consts.tile([125, 4, 50], F32)
            nc.gpsimd.dma_start(out=s_poswT, in_=d_poswT[:, :, :])
            s_z2 = consts.tile([125, 64], BF16)
            nc.gpsimd.dma_start(out=s_z2, in_=d_z2[:, :])
            s_ones = consts.tile([125, 128], BF16)
            nc.gpsimd.dma_start(out=s_ones, in_=d_ones[:, :])
            s_w1t = consts.tile([128, 500, 16], BF16)
            nc.gpsimd.dma_start(out=s_w1t, in_=d_w1t[:, :, :])
            s_b1t = consts.tile([125, 4], F32)
            nc.gpsimd.dma_start(out=s_b1t, in_=d_b1t[:, :])
            s_w2t = consts.tile([125, 4, 10], BF16)
            nc.gpsimd.dma_start(out=s_w2t, in_=d_w2t[:, :, :])
            s_b2 = consts.tile([10, 1], F32)
            nc.gpsimd.dma_start(out=s_b2, in_=d_b2[:, :])
            s_ng = consts.tile([128, 1], F32)
            nc.gpsimd.dma_start(out=s_ng, in_=d_ng[:, :])
            s_beta = consts.tile([128, 1], F32)
            nc.gpsimd.dma_start(out=s_beta, in_=d_beta[:, :])
            s_id10 = consts.tile([10, 10], F32)
            nc.gpsimd.dma_start(out=s_id10, in_=d_id10[:, :])
            eps_t = consts.tile([128, 1], F32)
            nc.vector.memset(eps_t, float(BN_EPS))

            # ---- tail (BN affine + FC + softmax), software-pipelined:
            # emitted one rep late so the AllReduce latency hides under the
            # next rep's adder instead of head-blocking the engine queues ----
            def emit_tail(h2f, ccp):
                gs = small.tile([128, 2], F32, tag="gs")
                nc.sync.dma_start(out=gs,
                                  in_=(ccp[0] if for_sim else ccp[1])[:, :])
                # rstd = exp(-0.5*ln(var+eps)): stays on one ACT table set
                inv_n = 1.0 / (B_TOTAL * 16)
                mean = small.tile([128, 1], F32, tag="mean")
                nc.vector.tensor_scalar(out=mean, in0=gs[:, 0:1],
                                        scalar1=inv_n, scalar2=None,
                                        op0=ALU.mult)
                msq = small.tile([128, 1], F32, tag="msq")
                nc.vector.tensor_tensor(out=msq, in0=mean, in1=mean,
                                        op=ALU.mult)
                var = small.tile([128, 1], F32, tag="var")
                nc.vector.scalar_tensor_tensor(
                    out=var, in0=gs[:, 1:2], scalar=inv_n, in1=msq,
                    op0=ALU.mult, op1=ALU.subtract)
                lnv = small.tile([128, 1], F32, tag="lnv")
                nc.scalar.activation(out=lnv, in_=var, func=ACTF.Ln,
                                     bias=eps_t, scale=1.0)
                rstd = small.tile([128, 1], F32, tag="rstd")
                nc.scalar.activation(out=rstd, in_=lnv, func=ACTF.Exp,
                                     scale=-0.5)
                scl = small.tile([128, 1], F32, tag="scl")
                nc.vector.tensor_tensor(out=scl, in0=rstd, in1=s_ng,
                                        op=ALU.mult)
                # bco2 = mean*scl - beta; h2bn = h2*scl - bco2
                bco2 = small.tile([128, 1], F32, tag="bco2")
                nc.vector.scalar_tensor_tensor(
                    out=bco2, in0=mean, scalar=scl, in1=s_beta,
                    op0=ALU.mult, op1=ALU.subtract)

                h2bn = small.tile([128, 4, 4, BC], BF16, tag="h2bn")
                nc.vector.tensor_scalar(
                    out=h2bn.rearrange("p i j b -> p (i j b)"), in0=h2f,
                    scalar1=scl, scalar2=bco2, op0=ALU.mult, op1=ALU.subtract)

                # ---- FC1 (+bias+relu) -> x2 [125, 32] x 4 ----
                x2 = []
                with tc.tile_pool(name="fcps", bufs=1, space="PSUM") as fcps:
                    for uc in range(4):
                        pu = fcps.tile([125, BC], F32, tag=f"fc1ps{uc}")
                        for ij in range(16):
                            nc.tensor.matmul(
                                pu, s_w1t[:, uc * 125:(uc + 1) * 125, ij],
                                h2bn[:, ij // 4, ij % 4, :],
                                start=(ij == 0), stop=(ij == 15))
                        xc = small.tile([125, BC], BF16, tag=f"x2_{uc}")
                        nc.scalar.activation(out=xc, in_=pu, func=ACTF.Relu,
                                             bias=s_b1t[:, uc:uc + 1],
                                             scale=1.0)
                        x2.append(xc)

                    # ---- FC2 -> logits [10, 32] ----
                    plg = fcps.tile([10, BC], F32, tag="fc2ps")
                    for uc in range(4):
                        nc.tensor.matmul(plg, s_w2t[:, uc, :], x2[uc],
                                         start=(uc == 0), stop=(uc == 3))
                    lg = small.tile([10, BC], F32, tag="lg")
                    nc.vector.tensor_scalar(out=lg, in0=plg, scalar1=s_b2,
                                            scalar2=None, op0=ALU.add)

                    # ---- transpose to [32, 10] + log_softmax ----
                    # logits are O(1) (randn-scaled weights): exp can't
                    # overflow, skip the max-subtraction
                    plt = fcps.tile([BC, 10], F32, tag="ltps")
                    nc.tensor.transpose(plt, lg, s_id10)
                    ex = small.tile([BC, 10], F32, tag="ex")
                    nc.scalar.activation(out=ex, in_=plt, func=ACTF.Exp)
                    sm = small.tile([BC, 1], F32, tag="sm")
                    nc.vector.tensor_reduce(out=sm, in_=ex,
                                            axis=mybir.AxisListType.X,
                                            op=ALU.add)
                    ls = small.tile([BC, 1], F32, tag="ls")
                    nc.scalar.activation(out=ls, in_=sm, func=ACTF.Ln)
                    yt = small.tile([BC, 10], F32, tag="yt")
                    nc.vector.tensor_scalar(out=yt, in0=plt, scalar1=ls,
                                            scalar2=None, op0=ALU.subtract)
                    nc.sync.dma_start(out=d_y[:, :], in_=yt)

            # ---- conv1 + pool1 -> h1 [20, 12, 12, 32] (I, J, b) ----
            x_t = d_x.ap().tensor
            pending = None
            for _rep in range(reps):
              h1 = big.tile([20, 12, 12, BC], BF16, tag="h1", name="h1")
              # conv rhs: ONE im2col tile for all row-chunks, 5 DMAs total
              rhs = big.tile([25, 24, 24, BC], BF16, tag="convrhs")
              for kh in range(5):
                  src = bass.AP(
                      tensor=x_t,
                      offset=kh * 28 * BC,
                      ap=[[BC, 5], [28 * BC, 24], [1, 24 * BC]],
                  )
                  eng = nc.sync if kh % 2 == 0 else nc.scalar
                  eng.dma_start(out=rhs[kh * 5:kh * 5 + 5], in_=src)
              with tc.tile_pool(name="convps", bufs=2, space="PSUM") as convps:
                  for ic in range(12):                 # chunks of 2 output rows
                      ps = convps.tile([20, 2, 24, BC], F32, tag="convps")
                      rfl = rhs[:, 2 * ic:2 * ic + 2].rearrange(
                          "p a b c -> p (a b c)")
                      pflat = ps.rearrange("p a b c -> p (a b c)")
                      for s0, s1 in ((0, 512), (512, 1024), (1024, 1536)):
                          nc.tensor.matmul(pflat[:, s0:s1], s_w1conv[:, :],
                                           rfl[:, s0:s1], start=True,
                                           stop=True)
                      # pool1: odd-j half to SBUF on ACT, maxes on DVE
                      pv = ps.rearrange("p a (j two) b -> p a j two b", two=2)
                      codd = work.tile([20, 2, 12, BC], F32, tag="convodd")
                      nc.scalar.activation(out=codd, in_=pv[:, :, :, 1, :],
                                           func=ACTF.Copy)
                      h1j = work.tile([20, 2, 12, BC], F32, tag="h1j")
                      nc.vector.tensor_tensor(out=h1j, in0=pv[:, :, :, 0, :],
                                              in1=codd, op=ALU.max)
                      nc.vector.tensor_tensor(out=h1[:, ic], in0=h1j[:, 0],
                                              in1=h1j[:, 1], op=ALU.max)

              # ---- patches [125, 8, 8, 32] x 4 chunks; p = (kh*5+kw)*5+c' --
              # kh-major emission: patch DMAs for row-window kh unblock as
              # soon as pool1 rows kh..kh+7 are done. sync HWDGE + gpsimd
              # SWDGE rings; the ACT ring stays free for compute.
              patches = [big.tile([125, 8, 8, BC], BF16, tag=f"patches{q}",
                                  name=f"patches{q}") for q in range(4)]
              for q in range(4):
                  for kh in range(5):
                      for kw in range(5):
                          m = kh * 5 + kw
                          # q0/q1 (needed first) on the sync HWDGE ring;
                          # 2/3 of q2/q3 on the gpsimd SWDGE ring, where the
                          # previous rep's collective sits ahead of them
                          idx = q * 25 + m
                          eng = (nc.sync if q < 2 or idx % 3 == 0
                                 else nc.gpsimd)
                          eng.dma_start(
                              out=patches[q][m * 5:m * 5 + 5],
                              in_=h1[5 * q:5 * q + 5, kh:kh + 8, kw:kw + 8, :],
                          )

              # ---- adder ----
              # ACT channels: psum[p(c)] += sum_k |patch_k + negw_k| (one-hot +1)
              # DVE channels: psum[p(c)] += sum_k patch_k - 2*sum_k min(.,w)
              #   (|a-b| = a + b - 2 min(a,b); sum_k w_k dropped -- BN cancels
              #    per-channel shifts). One-hot reduces are 4-way column-tiled.
              with tc.tile_pool(name="addps", bufs=1, space="PSUM") as addps:
                  psum_add = addps.tile([128, 2048], F32)
                  for q in range(4):
                      pq = patches[q].rearrange("p i j b -> p (i j b)")
                      for nb in range(4):
                          nc.tensor.matmul(
                              psum_add[:, nb * 512:(nb + 1) * 512],
                              s_ones[:, :],
                              pq[:, nb * 512:(nb + 1) * 512],
                              start=(q == 0), stop=False)
                      for r in range(13):
                          chans = [(j, 4 * r + j) for j in range(4)
                                   if 4 * r + j < 50]
                          tiles = {}
                          for j, c in chans:
                              a = absp.tile([125, 2048], BF16, tag="abs")
                              if c in ACT_CH:
                                  nc.scalar.activation(
                                      out=a, in_=pq, func=ACTF.Abs,
                                      bias=s_negwT[:, q, c:c + 1], scale=1.0)
                                  lhsT = s_onehot[:, 32 - r:64 - r]
                              else:
                                  nc.vector.tensor_scalar(
                                      out=a, in0=pq,
                                      scalar1=s_poswT[:, q, c:c + 1],
                                      scalar2=None, op0=ALU.min)
                                  lhsT = s_z2[:, 32 - r:64 - r]
                              tiles[j] = (a, lhsT)
                          for nb in range(4):
                              for ji, (j, c) in enumerate(chans):
                                  a, lhsT = tiles[j]
                                  stop = (q == 3 and r == LAST_ROUND[j]
                                          and nb == 3)
                                  # sim_coltile: timing-only model of the
                                  # 4-way col-tile concurrency (cost model
                                  # charges each MM serially) - stub all but
                                  # one MM per concurrent group
                                  w = 512 if (not sim_coltile or ji == 0) else 8
                                  nc.tensor.matmul(
                                      psum_add[32 * j:32 * j + 32,
                                               nb * 512:nb * 512 + w],
                                      lhsT, a[:, nb * 512:nb * 512 + w],
                                      start=False, stop=stop,
                                      tile_position=(0, 32 * j))

                  # ---- pool2 (min) -> h2m [128, 4, 4, 32] ----
                  pv = psum_add.rearrange("p (i j two b) -> p i j two b",
                                          i=8, j=4, two=2)
                  co2 = small.tile([128, 8, 4, BC], F32)
                  nc.scalar.activation(out=co2, in_=pv[:, :, :, 1, :],
                                       func=ACTF.Copy)
                  h2j = small.tile([128, 8, 4, BC], F32)
                  nc.vector.tensor_tensor(out=h2j, in0=pv[:, :, :, 0, :],
                                          in1=co2, op=ALU.min)
                  h2v = h2j.rearrange("p (i two) j b -> p i two j b", two=2)
                  h2m = xrep.tile([128, 4, 4, BC], F32, tag="h2m")
                  nc.vector.tensor_tensor(out=h2m, in0=h2v[:, :, 0],
                                          in1=h2v[:, :, 1], op=ALU.min)

              h2f = h2m.rearrange("p i j b -> p (i j b)")      # [128, 512]

              # ---- BN stats + AllReduce ----
              ccp = (cc_ins[_rep % 2], cc_outs[_rep % 2])
              stats = small.tile([128, 2], F32, tag="stats")
              nc.vector.tensor_reduce(out=stats[:, 0:1], in_=h2f,
                                      axis=mybir.AxisListType.X, op=ALU.add)
              junk = small.tile([128, 512], F32, tag="junk")
              nc.vector.tensor_tensor(out=junk, in0=h2f, in1=h2f, op=ALU.mult)
              nc.vector.tensor_reduce(out=stats[:, 1:2], in_=junk,
                                      axis=mybir.AxisListType.X, op=ALU.add)
              nc.sync.dma_start(out=ccp[0][:, :], in_=stats)
              if not for_sim:
                  nc.gpsimd.collective_compute(
                      "AllReduce", ALU.add,
                      replica_groups=[list(range(N_CORES))],
                      ins=[ccp[0].ap().opt()], outs=[ccp[1].ap().opt()])

              if pending is not None:
                  emit_tail(*pending)
              pending = (h2f, ccp)

            emit_tail(*pending)

    return _patch_nc(nc)


_NC_CACHE = None


def _get_nc():
    global _NC_CACHE
    if _NC_CACHE is None:
        _NC_CACHE = build_nc()
    return _NC_CACHE


def make_in_maps(inputs):
    p = host_prep(inputs)
    shared = {k: p[k] for k in ("w1conv", "negwT", "poswT", "onehot32",
                                "z2_32", "ones128", "w1t", "b1t", "w2t", "b2",
                                "neg_gamma", "beta", "ident10")}
    import ml_dtypes
    return [
        {"x_imT": np.ascontiguousarray(
            p["x_im"][c * BC:(c + 1) * BC].T).astype(ml_dtypes.bfloat16),
         **shared}
        for c in range(N_CORES)
    ]


def kernel(**inputs) -> np.ndarray:
    nc = _get_nc()
    in_maps = make_in_maps(inputs)
    res = run_bass_kernel_spmd(nc, in_maps, core_ids=list(range(N_CORES)),
                               trace=False)
    return np.concatenate([res.results[c]["y"] for c in range(N_CORES)],
                          axis=0).astype(np.float32)


# revision 20
# speedup vs baseline: 1.0779x; 1.0779x over previous
"""Trainium2 Bass kernel for nn_Net_72447508349482 (dense_cnn).

Pipeline per core (batch sharded 256 -> 8 x 32):
  conv1 (PE matmul over im2col, rhs laid out [J,b,i,j]) -> maxpool2 via one
  DVE XY-reduce (or ACT-assisted copy+max for some row-chunks, for engine
  balance) -> adder2d: |patch - w| via ACT activation(Abs, bias) for ~1/4 of
  the channels and min(patch, w) on DVE tensor_scalar for the rest;
  partition-reduce over k via PE one-hot matmuls, 4-way COLUMN-TILED
  (tile_position=(0,32j)) so four channels' reduces run concurrently in the
  128x128 array. Channels live at permuted PSUM partitions
  p(c) = 32*(c%4) + c//4 (undone for free on the host via permuted FC1/BN
  parameters). -> minpool (= -maxpool(-x), sign folded into BN affine)
  -> BN batch stats with cross-core AllReduce of [128,2] sums
  (rstd via exp(-0.5*ln(var+eps)) keeps ACT on one table set)
  -> FC1 + ReLU -> FC2 -> log_softmax.

conv1 bias is folded into the adder weights on the host:
  maxpool(conv+b) = maxpool(conv)+b and |p + b - w| = |p - (w - b)|.

Self-contained: includes the BIR multi-wait splitting fix required by this
container's walrus build (rejects >1 sync wait per instruction).
"""

import json

import numpy as np

import concourse.bass as bass
import concourse.mybir as mybir
import concourse.tile as tile
from concourse.bass_utils import run_bass_kernel_spmd

N_CORES = 8
B_TOTAL = 256
BC = B_TOTAL // N_CORES          # 32 per core
BN_EPS = 1e-5
F32 = mybir.dt.float32
BF16 = mybir.dt.bfloat16
ALU = mybir.AluOpType
ACTF = mybir.ActivationFunctionType

# channel -> psum partition: four col-tile groups j = c % 4, row r = c // 4
PERM = [32 * (c % 4) + c // 4 for c in range(50)]
# channels whose |p-w| runs on ACT (others: min(p,w) on DVE)
ACT_CH = {c for c in range(50) if c % 4 == 3} - {47}
LAST_ROUND = [12, 12, 11, 11]    # last round with a channel, per group


# --------------------------------------------------------------------------
# BIR post-processing: split multi-wait instructions (walrus limit = 1).
# --------------------------------------------------------------------------
def _fix_bir_bytes(raw: bytes) -> bytes:
    d = json.loads(raw)
    for fn in d.get("functions", []):
        for b in fn.get("blocks", []):
            insts = b.get("instructions", [])
            i = 0
            while i < len(insts):
                ins = insts[i]
                si = ins.get("sync_info") or {}
                w = si.get("on_wait") or []
                if len(w) > 1:
                    for j, extra in enumerate(w[:-1]):
                        insts.insert(i, {
                            "name": f"{ins['name']}_wsplit{j}",
                            "opcode": "EventSemaphore",
                            "engine": ins["engine"],
                            "ins": [], "outs": [],
                            "debug": ins.get("debug", 0),
                            "sync_info": {"on_update": [], "on_wait": [extra]},
                        })
                        i += 1
                    si["on_wait"] = w[-1:]
                i += 1
    return json.dumps(d).encode()


def _patch_nc(nc):
    cls_fn = type(nc).to_json_bytes
    nc.to_json_bytes = lambda: _fix_bir_bytes(cls_fn(nc))
    return nc


# --------------------------------------------------------------------------
# Host-side input prep (pure rearranges/folds of the given parameters).
# --------------------------------------------------------------------------
def host_prep(inputs):
    import ml_dtypes
    bf = ml_dtypes.bfloat16
    f = lambda k: np.ascontiguousarray(np.asarray(inputs[k], np.float32))
    x = f("x")                                       # [256,1,28,28]
    conv1_w, conv1_b, adder_w = f("conv1_w"), f("conv1_b"), f("adder_w")
    p = {}
    p["x_im"] = x.reshape(B_TOTAL, 784)
    wfold = adder_w - conv1_b[None, :, None, None]
    A = (-wfold.reshape(50, 20, 25))                 # [o, c, m]
    # chunk q: channels 5q..5q+4; within-chunk partition p = m*5 + c'
    negwT = np.stack(
        [A[:, 5 * q:5 * q + 5, :].transpose(2, 1, 0).reshape(125, 50)
         for q in range(4)], axis=1)                 # [125, 4, 50]
    p["negwT"] = np.ascontiguousarray(negwT)
    poswT = np.stack(
        [(-A)[:, 5 * q:5 * q + 5, :].transpose(2, 1, 0).reshape(125, 50)
         for q in range(4)], axis=1)                 # [125, 4, 50] = +w'
    p["poswT"] = np.ascontiguousarray(poswT)
    p["w1conv"] = np.ascontiguousarray(conv1_w.reshape(20, 25).T).astype(bf)
    oh = np.zeros((125, 64), bf)
    oh[:, 32] = bf(1.0)
    p["onehot32"] = oh
    z2 = np.zeros((125, 64), bf)
    z2[:, 32] = bf(-2.0)
    p["z2_32"] = z2
    ones128 = np.zeros((125, 128), bf)
    for c in range(50):
        if c not in ACT_CH:
            ones128[:, PERM[c]] = bf(1.0)
    p["ones128"] = ones128
    w1 = f("fc1_w").reshape(500, 50, 16)             # [u, c, ij]
    w1p = np.zeros((128, 500, 16), np.float32)
    for c in range(50):
        w1p[PERM[c]] = w1[:, c, :]
    p["w1t"] = np.ascontiguousarray(w1p).astype(bf)  # [128,500,16]
    p["b1t"] = np.ascontiguousarray(f("fc1_b").reshape(4, 125).T)  # [125,4]
    p["w2t"] = np.ascontiguousarray(
        f("fc2_w").T.reshape(4, 125, 10).transpose(1, 0, 2)).astype(bf)
    p["b2"] = f("fc2_b").reshape(10, 1)
    ng = np.zeros((128, 1), np.float32)
    bt = np.zeros((128, 1), np.float32)
    g, b_ = f("bn_gamma"), f("bn_beta")
    for c in range(50):
        ng[PERM[c], 0] = -g[c]
        bt[PERM[c], 0] = b_[c]
    p["neg_gamma"] = ng
    p["beta"] = bt
    p["ident10"] = np.eye(10, dtype=np.float32)
    return p


# --------------------------------------------------------------------------
# Device program.
# --------------------------------------------------------------------------
def build_nc(reps: int = 1, for_sim: bool = False, sim_coltile: bool = False):
    nc = bass.Bass("TRN2", target_bir_lowering=False, debug=False,
                   num_devices=1 if for_sim else N_CORES)
    d_x = nc.dram_tensor("x_imT", [784, BC], BF16, kind="ExternalInput")
    d_w1conv = nc.dram_tensor("w1conv", [25, 20], BF16, kind="ExternalInput")
    d_negwT = nc.dram_tensor("negwT", [125, 4, 50], F32, kind="ExternalInput")
    d_onehot = nc.dram_tensor("onehot32", [125, 64], BF16, kind="ExternalInput")
    d_poswT = nc.dram_tensor("poswT", [125, 4, 50], F32, kind="ExternalInput")
    d_z2 = nc.dram_tensor("z2_32", [125, 64], BF16, kind="ExternalInput")
    d_ones = nc.dram_tensor("ones128", [125, 128], BF16, kind="ExternalInput")
    d_w1t = nc.dram_tensor("w1t", [128, 500, 16], BF16, kind="ExternalInput")
    d_b1t = nc.dram_tensor("b1t", [125, 4], F32, kind="ExternalInput")
    d_w2t = nc.dram_tensor("w2t", [125, 4, 10], BF16, kind="ExternalInput")
    d_b2 = nc.dram_tensor("b2", [10, 1], F32, kind="ExternalInput")
    d_ng = nc.dram_tensor("neg_gamma", [128, 1], F32, kind="ExternalInput")
    d_beta = nc.dram_tensor("beta", [128, 1], F32, kind="ExternalInput")
    d_id10 = nc.dram_tensor("ident10", [10, 10], F32, kind="ExternalInput")
    d_y = nc.dram_tensor("y", [BC, 10], F32, kind="ExternalOutput")
    cc_ins = [nc.dram_tensor(f"cc_in{i}", [128, 2], F32) for i in range(2)]
    cc_outs = [nc.dram_tensor(f"cc_out{i}", [128, 2], F32,
                              addr_space="Shared") for i in range(2)]

    with tile.TileContext(nc) as tc:
        with (
            tc.tile_pool(name="consts", bufs=1) as consts,
            tc.tile_pool(name="big", bufs=1) as big,
            tc.tile_pool(name="work", bufs=3) as work,
            tc.tile_pool(name="absp", bufs=16) as absp,
            tc.tile_pool(name="small", bufs=1) as small,
            tc.tile_pool(name="xrep", bufs=2) as xrep,
        ):
            # ---- load constants (gpsimd SWDGE ring: keeps the sync/scalar
            # HWDGE rings free for the conv feed) ----
            s_w1conv = consts.tile([25, 20], BF16)
            nc.sync.dma_start(out=s_w1conv, in_=d_w1conv[:, :])
            s_negwT = consts.tile([125, 4, 50], F32)
            nc.gpsimd.dma_start(out=s_negwT, in_=d_negwT[:, :, :])
            s_onehot = consts.tile([125, 64], BF16)
            nc.gpsimd.dma_start(out=s_onehot, in_=d_onehot[:, :])
            s_poswT = consts.tile([125, 4, 50], F32)
            nc.gpsimd.dma_start(out=s_poswT, in_=d_poswT[:, :, :])
            s_z2 = consts.tile([125, 64], BF16)
            nc.gpsimd.dma_start(out=s_z2, in_=d_z2[:, :])
            s_ones = consts.tile([125, 128], BF16)
            nc.gpsimd.dma_start(out=s_ones, in_=d_ones[:, :])
            s_w1t = consts.tile([128, 500, 16], BF16)
            nc.gpsimd.dma_start(out=s_w1t, in_=d_w1t[:, :, :])
            s_b1t = consts.tile([125, 4], F32)
            nc.gpsimd.dma_start(out=s_b1t, in_=d_b1t[:, :])
            s_w2t = consts.tile([125, 4, 10], BF16)
            nc.gpsimd.dma_start(out=s_w2t, in_=d_w2t[:, :, :])
            s_b2 = consts.tile([10, 1], F32)
            nc.gpsimd.dma_start(out=s_b2, in_=d_b2[:, :])
            s_ng = consts.tile([128, 1], F32)
            nc.gpsimd.dma_start(out=s_ng, in_=d_ng[:, :])
            s_beta = consts.tile([128, 1], F32)
            nc.gpsimd.dma_start(out=s_beta, in_=d_beta[:, :])
            s_id10 = consts.tile([10, 10], F32)
            nc.gpsimd.dma_start(out=s_id10, in_=d_id10[:, :])
            eps_t = consts.tile([128, 1], F32)
            nc.vector.memset(eps_t, float(BN_EPS))

            # ---- tail (BN affine + FC + softmax), software-pipelined:
            # emitted one rep late so the AllReduce latency hides under the
            # next rep's adder instead of head-blocking the engine queues ----
            def emit_tail(h2f, ccp):
                gs = small.tile([128, 2], F32, tag="gs")
                nc.sync.dma_start(out=gs,
                                  in_=(ccp[0] if for_sim else ccp[1])[:, :])
                # rstd = exp(-0.5*ln(var+eps)): stays on one ACT table set
                inv_n = 1.0 / (B_TOTAL * 16)
                mean = small.tile([128, 1], F32, tag="mean")
                nc.vector.tensor_scalar(out=mean, in0=gs[:, 0:1],
                                        scalar1=inv_n, scalar2=None,
                                        op0=ALU.mult)
                msq = small.tile([128, 1], F32, tag="msq")
                nc.vector.tensor_tensor(out=msq, in0=mean, in1=mean,
                                        op=ALU.mult)
                var = small.tile([128, 1], F32, tag="var")
                nc.vector.scalar_tensor_tensor(
                    out=var, in0=gs[:, 1:2], scalar=inv_n, in1=msq,
                    op0=ALU.mult, op1=ALU.subtract)
                lnv = small.tile([128, 1], F32, tag="lnv")
                nc.scalar.activation(out=lnv, in_=var, func=ACTF.Ln,
                                     bias=eps_t, scale=1.0)
                rstd = small.tile([128, 1], F32, tag="rstd")
                nc.scalar.activation(out=rstd, in_=lnv, func=ACTF.Exp,
                                     scale=-0.5)
                scl = small.tile([128, 1], F32, tag="scl")
                nc.vector.tensor_tensor(out=scl, in0=rstd, in1=s_ng,
                                        op=ALU.mult)
                # bco2 = mean*scl - beta; h2bn = h2*scl - bco2
                bco2 = small.tile([128, 1], F32, tag="bco2")
                nc.vector.scalar_tensor_tensor(
                    out=bco2, in0=mean, scalar=scl, in1=s_beta,
                    op0=ALU.mult, op1=ALU.subtract)

                h2bn = small.tile([128, 4, 4, BC], BF16, tag="h2bn")
                nc.vector.tensor_scalar(
                    out=h2bn.rearrange("p i j b -> p (i j b)"), in0=h2f,
                    scalar1=scl, scalar2=bco2, op0=ALU.mult, op1=ALU.subtract)

                # ---- FC1 (+bias+relu) -> x2 [125, 32] x 4 ----
                x2 = []
                with tc.tile_pool(name="fcps", bufs=1, space="PSUM") as fcps:
                    for uc in range(4):
                        pu = fcps.tile([125, BC], F32, tag=f"fc1ps{uc}")
                        for ij in range(16):
                            nc.tensor.matmul(
                                pu, s_w1t[:, uc * 125:(uc + 1) * 125, ij],
                                h2bn[:, ij // 4, ij % 4, :],
                                start=(ij == 0), stop=(ij == 15))
                        xc = small.tile([125, BC], BF16, tag=f"x2_{uc}")
                        nc.scalar.activation(out=xc, in_=pu, func=ACTF.Relu,
                                             bias=s_b1t[:, uc:uc + 1],
                                             scale=1.0)
                        x2.append(xc)

                    # ---- FC2 -> logits [10, 32] ----
                    plg = fcps.tile([10, BC], F32, tag="fc2ps")
                    for uc in range(4):
                        nc.tensor.matmul(plg, s_w2t[:, uc, :], x2[uc],
                                         start=(uc == 0), stop=(uc == 3))
                    lg = small.tile([10, BC], F32, tag="lg")
                    nc.vector.tensor_scalar(out=lg, in0=plg, scalar1=s_b2,
                                            scalar2=None, op0=ALU.add)

                    # ---- transpose to [32, 10] + log_softmax ----
                    # logits are O(1) (randn-scaled weights): exp can't
                    # overflow, skip the max-subtraction
                    plt = fcps.tile([BC, 10], F32, tag="ltps")
                    nc.tensor.transpose(plt, lg, s_id10)
                    ex = small.tile([BC, 10], F32, tag="ex")
                    nc.scalar.activation(out=ex, in_=plt, func=ACTF.Exp)
                    sm = small.tile([BC, 1], F32, tag="sm")
                    nc.vector.tensor_reduce(out=sm, in_=ex,
                                            axis=mybir.AxisListType.X,
                                            op=ALU.add)
                    ls = small.tile([BC, 1], F32, tag="ls")
                    nc.scalar.activation(out=ls, in_=sm, func=ACTF.Ln)
                    yt = small.tile([BC, 10], F32, tag="yt")
                    nc.vector.tensor_scalar(out=yt, in0=plt, scalar1=ls,
                                            scalar2=None, op0=ALU.subtract)
                    nc.sync.dma_start(out=d_y[:, :], in_=yt)

            # ---- conv1 + pool1 -> h1 [20, 12, 12, 32] (I, J, b) ----
            x_t = d_x.ap().tensor
            pending = None
            for _rep in range(reps):
              h1 = big.tile([20, 12, 12, BC], BF16, tag="h1", name="h1")
              # conv rhs: ONE im2col tile for all row-chunks, 5 DMAs total
              rhs = big.tile([25, 24, 24, BC], BF16, tag="convrhs")
              for kh in range(5):
                  src = bass.AP(
                      tensor=x_t,
                      offset=kh * 28 * BC,
                      ap=[[BC, 5], [28 * BC, 24], [1, 24 * BC]],
                  )
                  eng = nc.sync if kh % 2 == 0 else nc.scalar
                  eng.dma_start(out=rhs[kh * 5:kh * 5 + 5], in_=src)
              with tc.tile_pool(name="convps", bufs=2, space="PSUM") as convps:
                  for ic in range(12):                 # chunks of 2 output rows
                      ps = convps.tile([20, 2, 24, BC], F32, tag="convps")
                      rfl = rhs[:, 2 * ic:2 * ic + 2].rearrange(
                          "p a b c -> p (a b c)")
                      pflat = ps.rearrange("p a b c -> p (a b c)")
                      for s0, s1 in ((0, 512), (512, 1024), (1024, 1536)):
                          nc.tensor.matmul(pflat[:, s0:s1], s_w1conv[:, :],
                                           rfl[:, s0:s1], start=True,
                                           stop=True)
                      # pool1: odd-j half to SBUF on ACT, maxes on DVE
                      pv = ps.rearrange("p a (j two) b -> p a j two b", two=2)
                      codd = work.tile([20, 2, 12, BC], F32, tag="convodd")
                      nc.scalar.activation(out=codd, in_=pv[:, :, :, 1, :],
                                           func=ACTF.Copy)
                      h1j = work.tile([20, 2, 12, BC], F32, tag="h1j")
                      nc.vector.tensor_tensor(out=h1j, in0=pv[:, :, :, 0, :],
                                              in1=codd, op=ALU.max)
                      nc.vector.tensor_tensor(out=h1[:, ic], in0=h1j[:, 0],
                                              in1=h1j[:, 1], op=ALU.max)

              # ---- patches [125, 8, 8, 32] x 4 chunks; p = (kh*5+kw)*5+c' --
              # kh-major emission: patch DMAs for row-window kh unblock as
              # soon as pool1 rows kh..kh+7 are done. sync HWDGE + gpsimd
              # SWDGE rings; the ACT ring stays free for compute.
              patches = [big.tile([125, 8, 8, BC], BF16, tag=f"patches{q}",
                                  name=f"patches{q}") for q in range(4)]
              for q in range(4):
                  for kh in range(5):
                      for kw in range(5):
                          m = kh * 5 + kw
                          # q0/q1 (needed first) on the sync HWDGE ring;
                          # 2/3 of q2/q3 on the gpsimd SWDGE ring, where the
                          # previous rep's collective sits ahead of them
                          idx = q * 25 + m
                          eng = (nc.sync if q < 2 or idx % 3 == 0
                                 else nc.gpsimd)
                          eng.dma_start(
                              out=patches[q][m * 5:m * 5 + 5],
                              in_=h1[5 * q:5 * q + 5, kh:kh + 8, kw:kw + 8, :],
                          )

              # ---- adder ----
              # ACT channels: psum[p(c)] += sum_k |patch_k + negw_k| (one-hot +1)
              # DVE channels: psum[p(c)] += sum_k patch_k - 2*sum_k min(.,w)
              #   (|a-b| = a + b - 2 min(a,b); sum_k w_k dropped -- BN cancels
              #    per-channel shifts). One-hot reduces are 4-way column-tiled.
              with tc.tile_pool(name="addps", bufs=1, space="PSUM") as addps:
                  psum_add = addps.tile([128, 2048], F32)
                  for q in range(4):
                      pq = patches[q].rearrange("p i j b -> p (i j b)")
                      for nb in range(4):
                          nc.tensor.matmul(
                              psum_add[:, nb * 512:(nb + 1) * 512],
                              s_ones[:, :],
                              pq[:, nb * 512:(nb + 1) * 512],
                              start=(q == 0), stop=False)
                      for r in range(13):
                          chans = [(j, 4 * r + j) for j in range(4)
                                   if 4 * r + j < 50]
                          tiles = {}
                          for j, c in chans:
                              a = absp.tile([125, 2048], BF16, tag="abs")
                              if c in ACT_CH:
                                  nc.scalar.activation(
                                      out=a, in_=pq, func=ACTF.Abs,
                                      bias=s_negwT[:, q, c:c + 1], scale=1.0)
                                  lhsT = s_onehot[:, 32 - r:64 - r]
                              else:
                                  nc.vector.tensor_scalar(
                                      out=a, in0=pq,
                                      scalar1=s_poswT[:, q, c:c + 1],
                                      scalar2=None, op0=ALU.min)
                                  lhsT = s_z2[:, 32 - r:64 - r]
                              tiles[j] = (a, lhsT)
                          for nb in range(4):
                              for ji, (j, c) in enumerate(chans):
                                  a, lhsT = tiles[j]
                                  stop = (q == 3 and r == LAST_ROUND[j]
                                          and nb == 3)
                                  # sim_coltile: timing-only model of the
                                  # 4-way col-tile concurrency (cost model
                                  # charges each MM serially) - stub all but
                                  # one MM per concurrent group
                                  w = 512 if (not sim_coltile or ji == 0) else 8
                                  nc.tensor.matmul(
                                      psum_add[32 * j:32 * j + 32,
                                               nb * 512:nb * 512 + w],
                                      lhsT, a[:, nb * 512:nb * 512 + w],
                                      start=False, stop=stop,
                                      tile_position=(0, 32 * j))

                  # ---- pool2 (min) -> h2m [128, 4, 4, 32] ----
                  pv = psum_add.rearrange("p (i j two b) -> p i j two b",
                                          i=8, j=4, two=2)
                  co2 = small.tile([128, 8, 4, BC], F32)
                  nc.scalar.activation(out=co2, in_=pv[:, :, :, 1, :],
                                       func=ACTF.Copy)
                  h2j = small.tile([128, 8, 4, BC], F32)
                  nc.vector.tensor_tensor(out=h2j, in0=pv[:, :, :, 0, :],
                                          in1=co2, op=ALU.min)
                  h2v = h2j.rearrange("p (i two) j b -> p i two j b", two=2)
                  h2m = xrep.tile([128, 4, 4, BC], F32, tag="h2m")
                  nc.vector.tensor_tensor(out=h2m, in0=h2v[:, :, 0],
                                          in1=h2v[:, :, 1], op=ALU.min)

              h2f = h2m.rearrange("p i j b -> p (i j b)")      # [128, 512]

              # ---- BN stats + AllReduce ----
              ccp = (cc_ins[_rep % 2], cc_outs[_rep % 2])
              stats = small.tile([128, 2], F32, tag="stats")
              nc.vector.tensor_reduce(out=stats[:, 0:1], in_=h2f,
                                      axis=mybir.AxisListType.X, op=ALU.add)
              junk = small.tile([128, 512], F32, tag="junk")
              nc.vector.tensor_tensor(out=junk, in0=h2f, in1=h2f, op=ALU.mult)
              nc.vector.tensor_reduce(out=stats[:, 1:2], in_=junk,
                                      axis=mybir.AxisListType.X, op=ALU.add)
              nc.sync.dma_start(out=ccp[0][:, :], in_=stats)
              if not for_sim:
                  nc.gpsimd.collective_compute(
                      "AllReduce", ALU.add,
                      replica_groups=[list(range(N_CORES))],
                      ins=[ccp[0].ap().opt()], outs=[ccp[1].ap().opt()])

              if pending is not None:
                  emit_tail(*pending)
              pending = (h2f, ccp)

            emit_tail(*pending)

    return _patch_nc(nc)


_NC_CACHE = None


def _get_nc():
    global _NC_CACHE
    if _NC_CACHE is None:
        _NC_CACHE = build_nc()
    return _NC_CACHE


def make_in_maps(inputs):
    p = host_prep(inputs)
    shared = {k: p[k] for k in ("w1conv", "negwT", "poswT", "onehot32",
                                "z2_32", "ones128", "w1t", "b1t", "w2t", "b2",
                                "neg_gamma", "beta", "ident10")}
    import ml_dtypes
    return [
        {"x_imT": np.ascontiguousarray(
            p["x_im"][c * BC:(c + 1) * BC].T).astype(ml_dtypes.bfloat16),
         **shared}
        for c in range(N_CORES)
    ]


def kernel(**inputs) -> np.ndarray:
    nc = _get_nc()
    in_maps = make_in_maps(inputs)
    res = run_bass_kernel_spmd(nc, in_maps, core_ids=list(range(N_CORES)),
                               trace=False)
    return np.concatenate([res.results[c]["y"] for c in range(N_CORES)],
                          axis=0).astype(np.float32)
